# revision 12
# baseline (speedup 1.0000x reference)
"""4-D average pool (kernel=2, stride=2) over [2,16,32,32,32,32] f32, on 8 NeuronCores.

Strategy: data-parallel over the 32 (b,c) slices -> 4 slices per core; the
per-core input is a contiguous [4096, 1024] f32 block (rows = (slice,d1,d2),
cols = (d3,d4)).

Each load tile covers 128 output rows (partition = pooled (d1,d2) output
index).  Two HWDGE sub-DMAs per tile (one per d1-pair element e1) gather the
rows; the d2-pair rows are adjacent in DRAM so they merge with the column
dim into contiguous 8 KiB runs.  The tile's free dim is then (e1, e2, d3,
d4) and ALL four poolings are chained DVE adds plus a 1/16 scale.  No
matmul, no PSUM, no ScalarE.
"""

import sys

import numpy as np

if "/opt/trn_rl_repo" not in sys.path:
    sys.path.insert(0, "/opt/trn_rl_repo")

import concourse.bacc as bacc
import concourse.bass as bass
import concourse.tile as tile
from concourse import mybir
from concourse.bass_utils import run_bass_kernel_spmd

N_CORES = 8
SLICES_PER_CORE = 4  # 32 (b,c) slices / 8 cores
ROWS = SLICES_PER_CORE * 1024  # 4096
N_TILES = 8  # one per 128 output rows; 2 MiB of input each
F32 = mybir.dt.float32


def build_nc() -> bass.Bass:
    # Bacc (not raw Bass): its compile() splits multi-sem sync waits into
    # event-semaphore instructions (TRN2 allows one wait per instruction).
    nc = bacc.Bacc()
    x = nc.dram_tensor("x", [ROWS, 1024], F32, kind="ExternalInput")
    y = nc.dram_tensor("y", [ROWS // 4, 256], F32, kind="ExternalOutput")

    # x row = 1024*s + 512*h + 64*o1p + 32*e1 + 2*o2 + e2 ; output row =
    # 256*s + 128*h + 16*o1p + o2  (d1 = 2*(8*h+o1p) + e1, d2 = 2*o2 + e2).
    xv = x[:].rearrange(
        "(s h o1p e1 o2 e2) f -> s h o1p e1 o2 e2 f",
        s=SLICES_PER_CORE, h=2, o1p=8, e1=2, o2=16, e2=2,
    )

    with tile.TileContext(nc) as tc:
        with (
            tc.tile_pool(name="inp", bufs=N_TILES) as inp,
            tc.tile_pool(name="m1p", bufs=4) as m1p,
            tc.tile_pool(name="m2p", bufs=4) as m2p,
            tc.tile_pool(name="obp", bufs=N_TILES) as obp,
        ):
            for l in range(N_TILES):
                s, h = l // 2, l % 2
                t = inp.tile([128, 4096], F32, tag="t")
                for e1 in range(2):
                    src = xv[s, h, :, e1, :, :, :]  # [o1p, o2, e2, f]
                    src = src.rearrange("o1p o2 e2 f -> o1p o2 (e2 f)")
                    nc.sync.dma_start(t[:, 2048 * e1 : 2048 * (e1 + 1)], src)

                # pool e1 (d1 pairs): [128, 2, 2048] -> [128, 2048]
                m1 = m1p.tile([128, 2048], F32, tag="m1")
                nc.vector.tensor_add(m1[:], t[:, 0:2048], t[:, 2048:4096])
                # pool e2 (d2 pairs): [128, 2, 1024] -> [128, 1024]
                m2 = m2p.tile([128, 1024], F32, tag="m2")
                nc.vector.tensor_add(m2[:], m1[:, 0:1024], m1[:, 1024:2048])
                # pool d4 pairs: [128, 32d3, 16o4, 2e4] -> [128, 512]
                m2v = m2[:].rearrange("p (d3 o4 e4) -> p d3 o4 e4", d3=32, o4=16)
                m3 = m2p.tile([128, 512], F32, tag="m3")
                m3v = m3[:].rearrange("p (d3 o4) -> p d3 o4", d3=32)
                nc.vector.tensor_add(m3v, m2v[:, :, :, 0], m2v[:, :, :, 1])
                # pool d3 pairs: [128, 16o3, 2e3, 16o4] -> [128, 256]
                m3w = m3[:].rearrange("p (o3 e3 o4) -> p o3 e3 o4", o3=16, e3=2)
                m4 = m2p.tile([128, 256], F32, tag="m4")
                m4v = m4[:].rearrange("p (o3 o4) -> p o3 o4", o3=16)
                nc.vector.tensor_add(m4v, m3w[:, :, 0, :], m3w[:, :, 1, :])
                # scale by 1/16 (DVE tensor_scalar runs 2x for fp32 SBUF)
                ob = obp.tile([128, 256], F32, tag="ob")
                nc.vector.tensor_scalar_mul(ob[:], m4[:], 1.0 / 16.0)
                nc.scalar.dma_start(y[128 * l : 128 * (l + 1), :], ob[:])

    nc.compile()
    return nc


_NC_CACHE: bass.Bass | None = None


def kernel(nd_tensor: np.ndarray, _trace: bool = False):
    global _NC_CACHE
    x = np.ascontiguousarray(np.asarray(nd_tensor, dtype=np.float32)).reshape(
        32, 1024, 1024
    )
    if _NC_CACHE is None:
        _NC_CACHE = build_nc()
    nc = _NC_CACHE

    in_maps = [
        {
            "x": np.ascontiguousarray(
                x[SLICES_PER_CORE * i : SLICES_PER_CORE * (i + 1)]
            ).reshape(ROWS, 1024)
        }
        for i in range(N_CORES)
    ]
    res = run_bass_kernel_spmd(
        nc, in_maps, core_ids=list(range(N_CORES)), trace=_trace
    )
    out = np.stack([res.results[i]["y"] for i in range(N_CORES)])  # [8,1024,256]
    out = out.reshape(2, 16, 16, 16, 16, 16).astype(np.float32)
    if _trace:
        kernel.last_results = res
    return out


# revision 14
# speedup vs baseline: 1.8639x; 1.8639x over previous
"""4-D average pool (kernel=2, stride=2) over [2,16,32,32,32,32] f32, on 8 NeuronCores.

Strategy: data-parallel over the 32 (b,c) slices -> 4 slices per core; the
per-core input is a contiguous [4096, 1024] f32 block (rows = (slice,d1,d2),
cols = (d3,d4)).

16 fully-contiguous 1 MiB loads (rows stay the partition dim - strided
gathers measure ~2x slower on HBM under 8-core load).  Per load tile
[128, 2048] (= 2 row-chunks x 1024 cols):
  - two DVE adds pool the free dim (d4 pairs, then d3 pairs) -> [128, 512]
  - ONE fp32 matmul with a constant [128, 32] pooling matrix (stationary
    weights, 32-column LDWEIGHTS is ~free) pools the (d1,d2) partition
    pairs for both chunks at once -> PSUM [32, 512]
  - ScalarE copies PSUM->SBUF; the store DMA scatters the two 32-row
    chunks to their output rows.
The 1/16 average scale is folded into the pooling matrix.
"""

import sys

import numpy as np

if "/opt/trn_rl_repo" not in sys.path:
    sys.path.insert(0, "/opt/trn_rl_repo")

import concourse.bacc as bacc
import concourse.bass as bass
import concourse.tile as tile
from concourse import mybir
from concourse.bass_utils import run_bass_kernel_spmd

N_CORES = 8
SLICES_PER_CORE = 4  # 32 (b,c) slices / 8 cores
ROWS = SLICES_PER_CORE * 1024  # 4096
N_LOADS = 16  # 1 MiB loads: 256 input rows (2 chunks of 128) each
F32 = mybir.dt.float32


def _build_pm() -> np.ndarray:
    # B[r, j] = 1/16 iff chunk row r = 32*d1l + d2 pools into chunk output
    # row j = 16*(d1l//2) + d2//2   (d1l in [0,4), d2 in [0,32))
    b = np.zeros((128, 32), np.float32)
    for d1l in range(4):
        for d2 in range(32):
            b[32 * d1l + d2, 16 * (d1l // 2) + d2 // 2] = 1.0 / 16.0
    return b


def build_nc() -> bass.Bass:
    # Bacc (not raw Bass): its compile() splits multi-sem sync waits into
    # event-semaphore instructions (TRN2 allows one wait per instruction).
    nc = bacc.Bacc()
    x = nc.dram_tensor("x", [ROWS, 1024], F32, kind="ExternalInput")
    pm = nc.dram_tensor("pm", [128, 32], F32, kind="ExternalInput")
    y = nc.dram_tensor("y", [ROWS // 4, 256], F32, kind="ExternalOutput")

    with tile.TileContext(nc) as tc:
        with (
            tc.tile_pool(name="pmp", bufs=1) as pmp,
            tc.tile_pool(name="inp", bufs=N_LOADS) as inp,
            tc.tile_pool(name="m1p", bufs=4) as m1p,
            tc.tile_pool(name="m2p", bufs=4) as m2p,
            tc.tile_pool(name="psp", bufs=8, space=bass.MemorySpace.PSUM) as psp,
            tc.tile_pool(name="obp", bufs=8) as obp,
        ):
            pm_t = pmp.tile([128, 32], F32)
            nc.sync.dma_start(pm_t[:], pm[:])

            for l in range(N_LOADS):
                # contiguous 1 MiB: input rows 256l .. 256(l+1), 2 chunks of
                # 128 rows side by side in the free dim
                t = inp.tile([128, 2048], F32, tag="t")
                src = x[256 * l : 256 * (l + 1), :].rearrange(
                    "(q p) c -> p q c", p=128
                )
                nc.sync.dma_start(t[:].rearrange("p (q c) -> p q c", q=2), src)

                # pool d4 pairs: [128, 2q, 32d3, 16o4, 2e4] -> [128, 1024]
                v = t[:].rearrange(
                    "p (q d3 o4 e4) -> p q d3 o4 e4", q=2, d3=32, o4=16
                )
                m1 = m1p.tile([128, 1024], F32, tag="m1")
                m1v = m1[:].rearrange("p (q d3 o4) -> p q d3 o4", q=2, d3=32)
                nc.vector.tensor_add(m1v, v[:, :, :, :, 0], v[:, :, :, :, 1])

                # pool d3 pairs: [128, 2q, 16o3, 2e3, 16o4] -> [128, 512]
                w = m1[:].rearrange(
                    "p (q o3 e3 o4) -> p q o3 e3 o4", q=2, o3=16, o4=16
                )
                m2 = m2p.tile([128, 512], F32, tag="m2")
                m2v = m2[:].rearrange("p (q o3 o4) -> p q o3 o4", q=2, o3=16)
                nc.vector.tensor_add(m2v, w[:, :, :, 0, :], w[:, :, :, 1, :])

                # pool (d1,d2) partition pairs for both chunks in one matmul
                ps = psp.tile([32, 512], F32, tag="ps")
                nc.tensor.matmul(ps[:], pm_t[:], m2[:], start=True, stop=True)

                ob = obp.tile([32, 512], F32, tag="ob")
                nc.scalar.copy(ob[:], ps[:])

                # chunk q lands at output rows 64l + 32q
                dst = y[64 * l : 64 * (l + 1), :].rearrange("(q j) c -> j q c", j=32)
                nc.scalar.dma_start(dst, ob[:].rearrange("j (q c) -> j q c", q=2))

    nc.compile()
    return nc


_NC_CACHE: bass.Bass | None = None


def kernel(nd_tensor: np.ndarray, _trace: bool = False):
    global _NC_CACHE
    x = np.ascontiguousarray(np.asarray(nd_tensor, dtype=np.float32)).reshape(
        32, 1024, 1024
    )
    pm = _build_pm()
    if _NC_CACHE is None:
        _NC_CACHE = build_nc()
    nc = _NC_CACHE

    in_maps = [
        {
            "x": np.ascontiguousarray(
                x[SLICES_PER_CORE * i : SLICES_PER_CORE * (i + 1)]
            ).reshape(ROWS, 1024),
            "pm": pm,
        }
        for i in range(N_CORES)
    ]
    res = run_bass_kernel_spmd(
        nc, in_maps, core_ids=list(range(N_CORES)), trace=_trace
    )
    out = np.stack([res.results[i]["y"] for i in range(N_CORES)])  # [8,1024,256]
    out = out.reshape(2, 16, 16, 16, 16, 16).astype(np.float32)
    if _trace:
        kernel.last_results = res
    return out
